# revision 13
# baseline (speedup 1.0000x reference)
"""AFT-Full (Attention-Free Transformer) distributed Bass kernel for 8 TRN2 NeuronCores.

Reference math (B=4, N=512, D=128):
    q = sigmoid(x @ Wq^T + bq); k = x @ Wk^T + bk; v = x @ Wv^T + bv
    s[b,t,j,d] = k[b,j,d] + pos_bias[t,j];  m = max_j s
    out = q * (sum_j exp(s-m) * v) / (sum_j exp(s-m))

Algebra used:
  * The max-stabilizer m cancels between numerator and denominator, and
    exp(k + pb) = exp(pb) * exp(k): with P = exp(pos_bias), ek = exp(k):
        out = q * (P @ (ek * v)) / (P @ ek)      (matmuls contract over j)
  * The k-bias cancels: exp(k+bk) = exp(bk)*exp(k) and exp(bk)[d] factors
    out of both j-sums, so k is projected WITHOUT bias.
  * The v-bias separates AFTER normalization:
        P @ (ek*(v0+bv)) = P @ (ek*v0) + (P @ ek) * bv = num0 + den*bv
    so v is projected WITHOUT bias and bv enters only in the epilogue:
        out = (num0 + den*bv) * recip(den * (1 + exp(-qlin-bq)))
    This removes the bv DMA + rank-1 matmuls from the k/v critical path.
  * sigmoid/q merge: 1/(1+exp(-qlin)) folded into the single reciprocal.

Sharding: 8 cores = 4 batches x 2 t-halves; no collectives. Each core gets
x[b]^T with its t-half's columns rotated to the front, and pos_bias rows
rotated identically, so the j-contraction order matches and one SPMD graph
serves all cores. Device computes out^T[d, t] for its (b, t-half).

Layout/perf notes:
  * pos_bias is cast to bf16 on host (layout prep): halves the largest DMA.
  * Only 2 input DMAs, both on the SP HWDGE ring in priority order
    (xw first: it gates the k/v matmuls). The Scalar queue carries only
    the ACT table load + activations + one output DMA half, so the first
    exp can fire as soon as k/v PSUM is ready.
  * PE warmup matmuls run while the loads are in flight (HAM clock gate),
    and a few dummy matmuls after the real work keep the PE sequencer
    clock high through the runtime's per-semaphore teardown (which is
    paced by the slowest engine - the PE - clearing ~52 semaphores).
  * Epilogue trim: keep ONLY the SP waits on the HWDGE completion
    semaphores (output-DMA receipt). The runtime's own teardown performs
    a full all-engine barrier and clears every semaphore 2..255, which
    subsumes Tile's gather/release barrier and its range-clear.
"""

import sys

import ml_dtypes
import numpy as np

try:
    import concourse.bass as bass
except ImportError:  # pragma: no cover
    sys.path.insert(0, "/opt/trn_rl_repo")
    import concourse.bass as bass

import concourse.mybir as mybir
import concourse.tile as tile
from concourse import bacc
from concourse.bass_utils import run_bass_kernel_spmd

F32 = mybir.dt.float32
BF16 = mybir.dt.bfloat16
B, N, D = 4, 512, 128
T = N // 2  # t-rows per core
JT = N // 128  # j tiles of 128
AF = mybir.ActivationFunctionType
ALU = mybir.AluOpType
N_PRE_MM = 3   # warmup matmuls while loads are in flight
N_POST_MM = 4  # dummy matmuls to keep the PE clock high into the teardown

# xw column layout, split into two DMA chunks:
#   chunk1 [Wk^T|Wv^T | x^T(rotated)] - gates the k/v matmuls, lands first
#   chunk2 [Wq^T | bq | bv]           - needed later (q matmul, eq, num2)
C_WKV = 0
C_X = 2 * D
C_WQ = C_X + N
C_BQ = C_WQ + D
C_BV = C_BQ + 1
XW = C_BV + 1


def build_nc() -> bass.Bass:
    # Bacc (not plain Bass): its compile() pass legalizes multi-wait
    # instructions (move_matmul_waits_to_ldweights, event semaphores),
    # which this walrus build requires.
    nc = bacc.Bacc()
    xw = nc.dram_tensor("xw", [D, XW], BF16, kind="ExternalInput")
    # pos_bias^T packed so each partition's 4 j-tiles are contiguous
    pbT = nc.dram_tensor("pbT", [128, JT * T], BF16, kind="ExternalInput")
    out = nc.dram_tensor("out", [D, T], F32, kind="ExternalOutput")

    with tile.TileContext(nc) as tc:
        with (
            tc.tile_pool(name="sb", bufs=1) as sb,
            tc.tile_pool(name="ps", bufs=1, space="PSUM") as ps,
        ):
            # ---- loads: SP HWDGE ring in priority order (FIFO per ring:
            # the k/v-gating chunk first, then the rest) ----
            xwb = sb.tile([D, XW], BF16, name="xwb")
            nc.sync.dma_start(xwb[:, 0:C_WQ], xw[:, 0:C_WQ])
            nc.sync.dma_start(xwb[:, C_WQ:XW], xw[:, C_WQ:XW])
            pbb = sb.tile([128, JT, T], BF16, name="pbb")
            pb3 = pbT[:].rearrange("p (j t) -> p j t", t=T)
            nc.sync.dma_start(pbb[:, 0:2, :], pb3[:, 0:2, :])
            nc.sync.dma_start(pbb[:, 2:4, :], pb3[:, 2:4, :])

            # ---- PE warmup while DMAs are in flight ----
            warm_in = sb.tile([128, N], BF16, name="warm_in")
            nc.vector.memset(warm_in[:], 0.0)
            warm_ps = ps.tile([128, N], F32, tag="warm_ps")
            for _ in range(N_PRE_MM):
                nc.tensor.matmul(
                    warm_ps[:], warm_in[:, 0:128], warm_in[:], start=True, stop=True
                )

            # -bq as a per-partition column for the eq ACT bias port
            bqn = sb.tile([D, 1], BF16, name="bqn")
            nc.vector.tensor_scalar_mul(bqn[:], xwb[:, C_BQ : C_BQ + 1], -1.0)
            bvf = sb.tile([D, 1], F32, name="bvf")
            nc.vector.tensor_copy(bvf[:], xwb[:, C_BV : C_BV + 1])

            wkv = xwb[:, C_WKV : C_WKV + 2 * D]
            xT = xwb[:, C_X : C_X + N]

            # ---- k/v projections (both biases algebraically removed) ----
            # kv[j][:, 0:128] = x_j @ Wk^T ; [:, 128:256] = x_j @ Wv^T
            kv_a = ps.tile([128, 2, 2 * D], F32, tag="kv_a")
            kv_b = ps.tile([128, 2, 2 * D], F32, tag="kv_b")
            kv_ps = [kv_a, kv_b]
            for j in range(JT):
                nc.tensor.matmul(
                    kv_ps[j // 2][:, j % 2, :],
                    xT[:, j * 128 : (j + 1) * 128],
                    wkv,
                    start=True,
                    stop=True,
                )
            # qlin^T[d,t] = Wq @ x[t-half]^T (bq applied in the eq ACT)
            q_ps = ps.tile([D, T], F32, tag="q_ps")
            nc.tensor.matmul(
                q_ps[:], xwb[:, C_WQ : C_WQ + D], xT[:, 0:T], start=True, stop=True
            )

            # ---- Scalar chain: ek (gates wt+den/num) first, eq last ----
            ek = sb.tile([128, JT, D], BF16, name="ek")
            wt = sb.tile([128, JT, D], BF16, name="wt")
            pt = sb.tile([128, JT, T], BF16, name="pt")
            nc.scalar.activation(ek[:, 0:2, :], kv_a[:, :, 0:D], AF.Exp)
            nc.vector.tensor_mul(wt[:, 0:2, :], ek[:, 0:2, :], kv_a[:, :, D : 2 * D])
            nc.scalar.activation(pt[:, 0:2, :], pbb[:, 0:2, :], AF.Exp)
            nc.scalar.activation(ek[:, 2:4, :], kv_b[:, :, 0:D], AF.Exp)
            nc.vector.tensor_mul(wt[:, 2:4, :], ek[:, 2:4, :], kv_b[:, :, D : 2 * D])
            nc.scalar.activation(pt[:, 2:4, :], pbb[:, 2:4, :], AF.Exp)
            eq = sb.tile([D, T], F32, name="eq")
            nc.scalar.activation(eq[:], q_ps[:], AF.Exp, scale=-1.0, bias=bqn[:])

            # ---- den^T = sum_j ek_j @ pt_j ; num^T = sum_j wt_j @ pt_j ----
            # den j01 first (den gates the f/rec chain), then interleave.
            den_ps = ps.tile([D, T], F32, tag="den_ps")
            num_ps = ps.tile([D, T], F32, tag="num_ps")
            nc.tensor.matmul(den_ps[:], ek[:, 0, :], pt[:, 0, :], start=True, stop=False)
            nc.tensor.matmul(den_ps[:], ek[:, 1, :], pt[:, 1, :], start=False, stop=False)
            nc.tensor.matmul(num_ps[:], wt[:, 0, :], pt[:, 0, :], start=True, stop=False)
            nc.tensor.matmul(num_ps[:], wt[:, 1, :], pt[:, 1, :], start=False, stop=False)
            nc.tensor.matmul(den_ps[:], ek[:, 2, :], pt[:, 2, :], start=False, stop=False)
            nc.tensor.matmul(den_ps[:], ek[:, 3, :], pt[:, 3, :], start=False, stop=True)
            nc.tensor.matmul(num_ps[:], wt[:, 2, :], pt[:, 2, :], start=False, stop=False)
            nc.tensor.matmul(num_ps[:], wt[:, 3, :], pt[:, 3, :], start=False, stop=True)

            # ---- epilogue ----
            # f = (eq + 1) * den ; rec = 1/f ; num' = den*bv + num
            # out^T = num' * rec, in halves with the two output DMAs on
            # different HWDGE rings so issue+receipt overlap.
            f = sb.tile([D, T], F32, name="f")
            nc.vector.scalar_tensor_tensor(
                f[:], eq[:], 1.0, den_ps[:], op0=ALU.add, op1=ALU.mult
            )
            rec = sb.tile([D, T], F32, name="rec")
            nc.vector.reciprocal_approx_fast(rec[:], f[:])
            # num2 = den*bv + num, via two DVE ops that each read only one
            # PSUM operand (hardware limit)
            dbv = sb.tile([D, T], F32, name="dbv")
            nc.vector.tensor_scalar_mul(dbv[:], den_ps[:], bvf[:])
            num2 = sb.tile([D, T], F32, name="num2")
            nc.vector.tensor_add(num2[:], dbv[:], num_ps[:])
            out_sb = sb.tile([D, T], F32, name="out_sb")
            half = T // 2
            # first half on DVE -> SP ring; second half on GpSimd ->
            # Activation ring: the muls and DMA issues run in parallel
            nc.vector.tensor_mul(out_sb[:, 0:half], rec[:, 0:half], num2[:, 0:half])
            nc.sync.dma_start(out[:, 0:half], out_sb[:, 0:half])
            nc.gpsimd.tensor_mul(out_sb[:, half:T], rec[:, half:T], num2[:, half:T])
            nc.scalar.dma_start(out[:, half:T], out_sb[:, half:T])

            # ---- post-work dummy matmuls: keep the PE sequencer clock
            # high through the runtime teardown (it clears ~52 semaphores
            # on the PE at a cadence set by the HAM-gated clock, pacing
            # the measured tail). Writing num_ps chains them AFTER its
            # epilogue readers so the scheduler cannot hoist them early.
            for _ in range(N_POST_MM):
                nc.tensor.matmul(
                    num_ps[:, 0:T], warm_in[:, 0:128], warm_in[:, 0:T],
                    start=True, stop=True,
                )

    _trim_prologue_barrier(nc)
    _trim_epilogue_barrier(nc)
    nc.finalize()
    return nc


def _trim_epilogue_barrier(nc):
    """Keep only the leading SP EventSemaphore waits (HWDGE completion =
    output-DMA receipt) from the Tile end block. The runtime's own
    teardown immediately follows with a per-engine drain, an all-engine
    barrier, and a full semaphore-file clear (2..255) - which subsumes
    Tile's gather/release barrier and its sem range-clear."""
    for f in nc.m.functions:
        for blk in f.blocks:
            if not blk.name.endswith("_end"):
                continue
            # Empty the block pre-finalize; Bacc's finalize/compile pass
            # then prepends the SP EventSemaphore waits on the HWDGE
            # completion semaphores (the output-DMA receipt), which is
            # the only part re-execution correctness needs.
            blk.instructions[:] = []


def _trim_prologue_barrier(nc):
    """Drop Bass.__init__'s const-AP barrier and the dead const memsets from
    the main block. The only live const (float32-0.0, the default ACT bias
    column) is written by GpSimd before any ACT reads it."""
    blk = nc.m.functions[0].blocks[0]
    keep = []
    for inst in blk.instructions:
        tn = type(inst).__name__
        if tn in ("InstDrain", "InstEventSemaphore"):
            continue
        if tn == "InstMemset":
            tgt = str(inst.outs[0].memref) if inst.outs else ""
            if "const-" in tgt and "float32-0" not in tgt:
                continue
        keep.append(inst)
    blk.instructions[:] = keep


def prepare_in_maps(x, Wq, bq, Wk, bk, Wv, bv, pos_bias):
    x = np.asarray(x, dtype=np.float32)
    pos_bias = np.asarray(pos_bias, dtype=np.float32)
    wkv = np.concatenate(
        [np.asarray(Wk, np.float32).T, np.asarray(Wv, np.float32).T], axis=1
    )
    wq_tail = np.concatenate(
        [
            np.asarray(Wq, np.float32).T,
            np.asarray(bq, np.float32)[:, None],
            np.asarray(bv, np.float32)[:, None],
        ],
        axis=1,
    )
    bk = np.asarray(bk, np.float32)  # unused on device: exp(bk) cancels

    in_maps = []
    for i in range(8):
        b, th = divmod(i, 2)
        t0 = th * T
        perm = np.concatenate([np.arange(t0, N), np.arange(0, t0)])
        xT = x[b][perm].T  # [128, 512]
        pb = pos_bias[t0 : t0 + T][:, perm].T  # [512, 256] (j, t)
        # pack so each SBUF partition's 4 j-tiles are contiguous: [128, 4*256]
        pb2 = np.ascontiguousarray(
            pb.reshape(JT, 128, T).transpose(1, 0, 2).reshape(128, JT * T)
        ).astype(ml_dtypes.bfloat16)
        in_maps.append(
            {
                "xw": np.ascontiguousarray(
                    np.concatenate([wkv, xT, wq_tail], axis=1).astype(
                        ml_dtypes.bfloat16
                    )
                ),
                "pbT": pb2,
            }
        )
    return in_maps


def assemble_output(results) -> np.ndarray:
    out = np.empty((B, N, D), np.float32)
    for i in range(8):
        b, th = divmod(i, 2)
        t0 = th * T
        out[b, t0 : t0 + T, :] = results[i]["out"].T
    return out


def kernel(x, Wq, bq, Wk, bk, Wv, bv, pos_bias) -> np.ndarray:
    in_maps = prepare_in_maps(x, Wq, bq, Wk, bk, Wv, bv, pos_bias)
    nc = build_nc()
    res = run_bass_kernel_spmd(nc, in_maps, core_ids=list(range(8))).results
    return assemble_output(res)


if __name__ == "__main__":
    rng = np.random.default_rng(0)
    s = 1.0 / np.sqrt(D)
    inputs = dict(
        x=rng.standard_normal((B, N, D), dtype=np.float32),
        Wq=rng.standard_normal((D, D), dtype=np.float32) * s,
        bq=rng.standard_normal((D,), dtype=np.float32) * s,
        Wk=rng.standard_normal((D, D), dtype=np.float32) * s,
        bk=rng.standard_normal((D,), dtype=np.float32) * s,
        Wv=rng.standard_normal((D, D), dtype=np.float32) * s,
        bv=rng.standard_normal((D,), dtype=np.float32) * s,
        pos_bias=rng.standard_normal((N, N), dtype=np.float32) * 0.1,
    )
    out = kernel(**inputs)
    print("kernel ran, out shape:", out.shape)
